# revision 8
# baseline (speedup 1.0000x reference)
"""Trainium2 Bass kernel for nn_Attention_xxc (dense transformer attention
with hop-distance bias). Data-parallel over batch: 8 cores x 2 batches.

Layout strategy (per core):
  - Host preps transposed inputs: xT [512, 2048], WqkvT [512, 1536] (q cols
    pre-scaled by 1/sqrt(hd)), WprojT [512, 512], biasT[h] = (alpha_h *
    sum_k w_hk Hstack_k).T in bf16.
  - qkv: q,k computed TRANSPOSED ([outch, tok], bf16), v computed NATURAL
    ([tok, vch], bf16) with a ones-column appended per head (65 cols/head).
  - scores computed transposed: S.T[m, n] = k_m . q_n + bias.T  (bias folded
    in via identity-matmul PSUM accumulation), exp on ACT -> P bf16.
  - AV: out_aug.T[d(+1), n] = v_aug.T @ P ; row 64 = softmax denominator.
  - normalize: broadcast 1/denom across partitions via K=1 matmul, multiply.
  - proj: y[n, o] = outT.T @ WprojT + bproj, natural layout, DMA out.
"""
import sys

sys.path.insert(0, "/opt/trn_rl_repo")

import numpy as np
import ml_dtypes

B, N, DIM = 16, 1024, 512
H, HD, KH = 8, 64, 5
SCALE = HD ** -0.5
NCORES = 8
BPC = B // NCORES          # batches per core
TOK = BPC * N              # tokens per core = 2048

_CACHE = {}


def _build():
    import concourse.bass as bass
    import concourse.bacc as bacc
    import concourse.mybir as mybir
    from concourse.tile import TileContext

    f32 = mybir.dt.float32
    f32r = mybir.dt.float32r
    bf16 = mybir.dt.bfloat16
    EXP = mybir.ActivationFunctionType.Exp
    CPY = mybir.ActivationFunctionType.Copy
    MUL = mybir.AluOpType.mult
    ADD = mybir.AluOpType.add

    nc = bacc.Bacc()
    xT = nc.declare_dram_parameter("xT", [DIM, TOK], bf16, isOutput=False)
    wqkvT = nc.declare_dram_parameter("wqkvT", [DIM, 3 * DIM], bf16, isOutput=False)
    wprojT = nc.declare_dram_parameter("wprojT", [DIM, DIM], bf16, isOutput=False)
    bprojb = nc.declare_dram_parameter("bprojb", [128, DIM], f32, isOutput=False)
    biasT = nc.declare_dram_parameter("biasT", [H, N, N], bf16, isOutput=False)
    eye = nc.declare_dram_parameter("eye", [128, 128], bf16, isOutput=False)
    ones64 = nc.declare_dram_parameter("ones64", [1, 64], bf16, isOutput=False)
    y = nc.declare_dram_parameter("y", [TOK, DIM], f32, isOutput=True)

    NT = TOK // 128            # 16 token tiles
    VW = H * (HD + 1)          # 520: v row width with ones col per head

    with TileContext(nc) as tc:
        with (
            tc.tile_pool(name="qk", bufs=1) as QK,
            tc.tile_pool(name="vres", bufs=1) as VR,
            tc.tile_pool(name="wp", bufs=1) as WP,
            tc.tile_pool(name="outT", bufs=1) as OT,
            tc.tile_pool(name="const", bufs=1) as CONST,
        ):
            eye_t = CONST.tile([128, 128], bf16, tag="eye", name="eye")
            nc.sync.dma_start(out=eye_t[:], in_=eye[:])
            ones_t = CONST.tile([1, 64], bf16, tag="ones", name="ones")
            nc.sync.dma_start(out=ones_t[:], in_=ones64[:])
            bpb_t = CONST.tile([128, DIM], f32, tag="bpb", name="bpb")
            nc.sync.dma_start(out=bpb_t[:], in_=bprojb[:])
            wp_t = [WP.tile([128, DIM], bf16, tag=f"wp{c}", name=f"wp{c}") for c in range(4)]
            for c in range(4):
                nc.sync.dma_start(out=wp_t[c][:], in_=wprojT[c * 128:(c + 1) * 128, :])

            qk_t = [QK.tile([128, TOK], bf16, tag=f"qk{o}", name=f"qk{o}") for o in range(8)]
            v_t = [VR.tile([128, VW], bf16, tag=f"v{t}", name=f"v{t}") for t in range(NT)]
            oT_t = [OT.tile([128, N], bf16, tag=f"oT{b}_{c}", name=f"oT{b}_{c}")
                    for b in range(BPC) for c in range(4)]

            # ---------------- phase 1: qkv projections ----------------
            with (
                tc.tile_pool(name="xw", bufs=1) as XW,
                tc.tile_pool(name="ps1", bufs=4, space="PSUM") as PS1,
            ):
                xT_t = [XW.tile([128, TOK], bf16, tag=f"x{c}", name=f"x{c}") for c in range(4)]
                wq_t = [XW.tile([128, 3 * DIM], bf16, tag=f"w{c}", name=f"w{c}") for c in range(4)]
                for c in range(4):
                    nc.sync.dma_start(out=xT_t[c][:], in_=xT[c * 128:(c + 1) * 128, :])
                    nc.sync.dma_start(out=wq_t[c][:], in_=wqkvT[c * 128:(c + 1) * 128, :])

                # q,k transposed: qkvT[o_tile, tok] ; o tiles 0..7 cover q,k
                for o in range(8):
                    for t in range(4):           # tok chunks of 512
                        ps = PS1.tile([128, 512], f32, tag="ps1", name="ps1")
                        for c in range(4):
                            nc.tensor.matmul(
                                ps[:], wq_t[c][:, o * 128:(o + 1) * 128],
                                xT_t[c][:, t * 512:(t + 1) * 512],
                                start=(c == 0), stop=(c == 3))
                        nc.vector.tensor_copy(qk_t[o][:, t * 512:(t + 1) * 512], ps[:])
                # v natural: [tok_tile, vch] -> packed per head with ones col
                for t in range(NT):
                    ps = PS1.tile([128, 512], f32, tag="ps1", name="ps1")
                    for c in range(4):
                        nc.tensor.matmul(
                            ps[:], xT_t[c][:, t * 128:(t + 1) * 128],
                            wq_t[c][:, 2 * DIM:3 * DIM],
                            start=(c == 0), stop=(c == 3))
                    dst = v_t[t][:, 0:VW].rearrange("p (h s) -> p h s", s=HD + 1)
                    nc.vector.tensor_copy(
                        dst[:, :, 0:HD],
                        ps[:].rearrange("p (h s) -> p h s", s=HD))
                    nc.vector.memset(dst[:, :, HD:HD + 1], 1.0)

            # ---------------- phase 2: attention ----------------
            with (
                tc.tile_pool(name="biasp", bufs=10) as BP,
                tc.tile_pool(name="pp", bufs=12) as PP,
                tc.tile_pool(name="nrm", bufs=4) as NRM,
                tc.tile_pool(name="ysb", bufs=3) as YSB,
                tc.tile_pool(name="pss", bufs=2, space="PSUM") as PSS,
                tc.tile_pool(name="pso", bufs=1, space="PSUM") as PSO,
                tc.tile_pool(name="psm", bufs=2, space="PSUM") as PSM,
            ):
                for h in range(H):
                    qt, po = qk_t[h // 2], (h % 2) * 64
                    kt = qk_t[4 + h // 2]
                    b_tiles = []
                    for mi in range(8):
                        bt = BP.tile([128, N], bf16, tag="bias", name="bias")
                        nc.sync.dma_start(
                            out=bt[:], in_=biasT[h, mi * 128:(mi + 1) * 128, :])
                        b_tiles.append(bt)
                    for b in range(BPC):
                        t0 = b * N
                        p_tiles = []
                        for mi in range(8):
                            ps = PSS.tile([128, N], f32, tag="pss", name="pss")
                            for nchunk in range(2):
                                sl = slice(nchunk * 512, (nchunk + 1) * 512)
                                nc.tensor.matmul(
                                    ps[:, sl],
                                    kt[po:po + 64, t0 + mi * 128: t0 + (mi + 1) * 128],
                                    qt[po:po + 64, t0 + nchunk * 512: t0 + (nchunk + 1) * 512],
                                    start=True, stop=False)
                                nc.tensor.matmul(
                                    ps[:, sl], eye_t[:], b_tiles[mi][:, sl],
                                    start=False, stop=True)
                            pt = PP.tile([128, N], bf16, tag="p", name="p")
                            nc.scalar.activation(pt[:], ps[:], EXP)
                            p_tiles.append(pt)
                        pso = PSO.tile([HD + 1, N], f32, tag="pso", name="pso")
                        for mi in range(8):
                            for nchunk in range(2):
                                sl = slice(nchunk * 512, (nchunk + 1) * 512)
                                nc.tensor.matmul(
                                    pso[:, sl],
                                    v_t[b * 8 + mi][:, h * (HD + 1):(h + 1) * (HD + 1)],
                                    p_tiles[mi][:, sl],
                                    start=(mi == 0), stop=(mi == 7))
                        # denominator -> broadcast -> reciprocal -> normalize
                        d_t = NRM.tile([1, N], bf16, tag="d", name="d")
                        nc.vector.tensor_copy(d_t[:], pso[64:65, :])
                        R_t = NRM.tile([64, N], f32, tag="R", name="R")
                        for nchunk in range(2):
                            sl = slice(nchunk * 512, (nchunk + 1) * 512)
                            psr = PSM.tile([64, 512], f32, tag="psm", name="psm")
                            nc.tensor.matmul(psr[:], ones_t[:], d_t[:, sl],
                                             start=True, stop=True)
                            nc.vector.reciprocal(R_t[:, sl], psr[:])
                        nc.vector.tensor_tensor(
                            oT_t[b * 4 + h // 2][po:po + 64, :],
                            pso[0:64, :], R_t[:], MUL)
                # ---------------- phase 3: output projection ----------------
                for b in range(BPC):
                    for t in range(8):
                        psy = PSM.tile([128, 512], f32, tag="psm", name="psm")
                        for c in range(4):
                            nc.tensor.matmul(
                                psy[:],
                                oT_t[b * 4 + c][:, t * 128:(t + 1) * 128],
                                wp_t[c][:], start=(c == 0), stop=(c == 3))
                        yt = YSB.tile([128, DIM], f32, tag="y", name="y")
                        nc.vector.tensor_tensor(yt[:], psy[:], bpb_t[:], ADD)
                        nc.sync.dma_start(
                            out=y[b * N + t * 128: b * N + (t + 1) * 128, :],
                            in_=yt[:])
    nc.compile()
    return nc


def _prep_host(x, Hstack, hop_logits_attn, rel_alpha, Wqkv, Wproj, bproj):
    bf = ml_dtypes.bfloat16
    lg = hop_logits_attn - hop_logits_attn.max(-1, keepdims=True)
    w = np.exp(lg)
    w /= w.sum(-1, keepdims=True)                      # [H, KH]
    Bh = np.einsum("hk,kij->hij", w.astype(np.float32),
                   Hstack.astype(np.float32))          # [H, N, N]
    biasT = np.ascontiguousarray(
        (rel_alpha[:, None, None] * Bh).transpose(0, 2, 1)).astype(bf)
    wqkvT = np.ascontiguousarray(Wqkv.T).astype(np.float32).copy()
    wqkvT[:, :DIM] *= SCALE                            # fold q scaling
    wqkvT = wqkvT.astype(bf)
    wprojT = np.ascontiguousarray(Wproj.T).astype(bf)
    bprojb = np.tile(bproj[None, :], (128, 1)).astype(np.float32)
    eye = np.eye(128, dtype=np.float32).astype(bf)
    ones64 = np.ones((1, 64), dtype=np.float32).astype(bf)
    shared = dict(wqkvT=wqkvT, wprojT=wprojT, bprojb=bprojb,
                  biasT=biasT, eye=eye, ones64=ones64)
    in_maps = []
    for i in range(NCORES):
        xi = x[i * BPC:(i + 1) * BPC].reshape(TOK, DIM)
        xTi = np.ascontiguousarray(xi.T).astype(bf)
        in_maps.append(dict(xT=xTi, **shared))
    return in_maps


def kernel(**inputs):
    from concourse.bass_utils import run_bass_kernel_spmd

    if "nc" not in _CACHE:
        _CACHE["nc"] = _build()
    nc = _CACHE["nc"]
    in_maps = _prep_host(
        np.asarray(inputs["x"], np.float32),
        np.asarray(inputs["Hstack"], np.float32),
        np.asarray(inputs["hop_logits_attn"], np.float32),
        np.asarray(inputs["rel_alpha"], np.float32),
        np.asarray(inputs["Wqkv"], np.float32),
        np.asarray(inputs["Wproj"], np.float32),
        np.asarray(inputs["bproj"], np.float32))
    res = run_bass_kernel_spmd(nc, in_maps, list(range(NCORES))).results
    out = np.concatenate([r["y"].reshape(BPC, N, DIM) for r in res], axis=0)
    return out.astype(np.float32)
